# revision 1
# baseline (speedup 1.0000x reference)
"""Trainium2 Bass kernel for nn_Dilateformer3 (multi-dilation local window attention).

Sharding: data-parallel over batch B=8 across 8 NeuronCores (1 image/core).

Per-core layout: channel-major ("c-major") activations [c_partitions, pixels].
 - padded K/V pixel grids 64x64 (real 56x56 at offset (+4,+4)) so all 9
   window shifts are plain free-dim offsets and out-of-image keys read 0.
 - scores s_j = sum_c Q*K_shift: DVE elementwise product + PE matmul with a
   block-diagonal ones lhsT reducing the 96 (2 heads x 48c) partitions to 2.
 - softmax: no masking needed -- zero-padded keys give score exactly 0 so
   exp=1; a host-precomputed invalid-neighbor count is subtracted from the
   denominator. exp on ACT, denominator born head-replicated via a second
   PE matmul (selden), normalize with DVE divide.
 - AV: P~ replicated to 96 c-lanes via PE sel matmul; DVE product with
   shifted V; DVE reduce over the 9 taps.
 - qkv + final proj: plain PE matmuls (bf16 in, fp32 PSUM accum).
"""

import sys
import numpy as np

for p in ("/opt/trn_rl_repo",):
    if p not in sys.path:
        sys.path.insert(0, p)

import ml_dtypes
import concourse.bass as bass
import concourse.bacc as bacc
import concourse.tile as tile
from concourse import mybir
from concourse.alu_op_type import AluOpType
from concourse.bass_utils import run_bass_kernel_spmd

BF16 = mybir.dt.bfloat16
F32 = mybir.dt.float32
NPBF16 = np.dtype(ml_dtypes.bfloat16)

B, H, W, C = 8, 56, 56, 288
NPIX = H * W                 # 3136
DILS = (1, 2, 3)
HD = 48                      # head dim
NBLK = 7                     # pixel blocks
BLK = 448                    # = 8 rows x 56 cols
PW = 64                      # padded width
PH = 64                      # padded height
NPAD = PW * PH               # 4096
PADR, PADC = 4, 4            # top/left pad
SCALE = HD ** -0.5
SHIFTS = [(a, b) for a in (-1, 0, 1) for b in (-1, 0, 1)]  # (kh-1, kw-1) order
N_CORES = 8


def build_nc():
    nc = bacc.Bacc("TRN2", target_bir_lowering=False)
    dt_in = {}

    def din(name, shape, dtype):
        t = nc.dram_tensor(name, shape, dtype, kind="ExternalInput")
        dt_in[name] = t
        return t

    xT = din("xT", [96, 3, NPIX], BF16)          # [cin%96, cin_chunk, px]
    wq = din("wq", [96, 9, 3, 96], BF16)         # [cin, m_chunk, k_chunk, cout]
    bq = din("bq", [96, 9], F32)                 # [cout%96, m_chunk]
    wp = din("wp", [96, 3, C], BF16)             # [cin%96, branch, cout]
    bp = din("bp", [128, C], F32)                # b_proj replicated over partitions
    ones18 = din("ones18", [96, 9, 18], BF16)    # per-tap head-sum reduction lhsT
    selden = din("selden", [18, 18], BF16)       # denominator replication lhsT
    selj = din("selj", [18, 9, 96], BF16)        # P~ -> 96-lane replication lhsT
    y = nc.dram_tensor("y", [NPIX, C], F32, kind="ExternalOutput")

    from contextlib import ExitStack
    with tile.TileContext(nc) as tc, ExitStack() as ctx:
        singles = ctx.enter_context(tc.tile_pool(name="singles", bufs=1))
        prodp = ctx.enter_context(tc.tile_pool(name="prodp", bufs=3))
        smallp = ctx.enter_context(tc.tile_pool(name="smallp", bufs=4))
        prjp = ctx.enter_context(tc.tile_pool(name="prjp", bufs=2))
        ppa = ctx.enter_context(tc.tile_pool(name="ppa", bufs=2, space="PSUM"))
        ppb = ctx.enter_context(tc.tile_pool(name="ppb", bufs=3, space="PSUM"))

        # ---- persistent SBUF tensors ----
        xT_sb = singles.tile([96, 3, NPIX], BF16)
        wq_sb = singles.tile([96, 9, 3, 96], BF16)
        bq_sb = singles.tile([96, 9], F32)
        wp_sb = singles.tile([96, 3, C], BF16)
        bp_sb = singles.tile([128, C], F32)
        ones18_sb = singles.tile([96, 9, 18], BF16)
        selden_sb = singles.tile([18, 18], BF16)
        selj_sb = singles.tile([18, 9, 96], BF16)
        Q_sb = singles.tile([96, 3, NPIX], BF16)
        K_sb = singles.tile([96, 3, NPAD], BF16)
        V_sb = singles.tile([96, 3, NPAD], BF16)
        Pc_sb = singles.tile([18, 3, NPIX], BF16)   # exp'd scores
        Pn_sb = singles.tile([18, 3, NPIX], BF16)   # normalized probs
        yb_sb = singles.tile([96, 3, NPIX], BF16)   # attention out (c-major)

        # ---- load inputs ----
        nc.sync.dma_start(out=xT_sb, in_=xT[:, :, :])
        nc.sync.dma_start(out=wq_sb, in_=wq[:, :, :, :])
        nc.sync.dma_start(out=bq_sb, in_=bq[:, :])
        nc.sync.dma_start(out=wp_sb, in_=wp[:, :, :])
        nc.sync.dma_start(out=bp_sb, in_=bp[:, :])
        nc.sync.dma_start(out=ones18_sb, in_=ones18[:, :, :])
        nc.sync.dma_start(out=selden_sb, in_=selden[:, :])
        nc.sync.dma_start(out=selj_sb, in_=selj[:, :, :])

        # ---- zero K/V pad regions ----
        for t in (K_sb, V_sb):
            for br in range(3):
                g = t[:, br, :].rearrange("p (r c) -> p r c", c=PW)
                nc.vector.memset(g[:, 0:PADR, :], 0.0)                 # top rows
                nc.vector.memset(g[:, PADR + H:PH, :], 0.0)            # bottom rows
                nc.vector.memset(g[:, PADR:PADR + H, 0:PADC], 0.0)     # left pad
                nc.vector.memset(g[:, PADR:PADR + H, PADC + W:PW], 0.0)  # right pad

        # Wait-absorbers: walrus limits sem waits per instruction; these tiny
        # reads make ACT/DVE observe the setup DMA + memset ticks once, so
        # the real evacuation ops only need the PE wait.
        warm_a = smallp.tile([1, 8], F32, tag="warma")
        nc.scalar.activation(out=warm_a, in_=V_sb[0:1, 2, 1980:1988],
                             func=mybir.ActivationFunctionType.Copy)
        warm_a2 = smallp.tile([1, 8], F32, tag="warma")
        nc.scalar.activation(out=warm_a2, in_=bq_sb[0:1, 0:8],
                             func=mybir.ActivationFunctionType.Copy)
        warm_v = smallp.tile([1, 8], F32, tag="warma")
        nc.vector.tensor_copy(warm_v, bq_sb[0:1, 0:8])
        warm_v2 = smallp.tile([1, 8], F32, tag="warma")
        nc.vector.tensor_copy(warm_v2, V_sb[0:1, 2, 1980:1988])

        # ---- phase 1: qkv projection ----
        for m in range(9):
            qkv_t, br = divmod(m, 3)  # 0=q 1=k 2=v
            for t in range(NBLK):
                ps = ppa.tile([96, BLK], F32, tag="mm96")
                for k in range(3):
                    nc.tensor.matmul(
                        ps,
                        lhsT=wq_sb[:, m, k, :],
                        rhs=xT_sb[:, k, t * BLK:(t + 1) * BLK],
                        start=(k == 0),
                        stop=(k == 2),
                    )
                if qkv_t == 0:
                    dest = Q_sb[:, br, t * BLK:(t + 1) * BLK]
                    src = ps
                else:
                    tgt = K_sb if qkv_t == 1 else V_sb
                    dest = tgt[:, br, :].rearrange("p (r c) -> p r c", c=PW)[
                        :, 8 * t + PADR:8 * t + 8 + PADR, PADC:PADC + W
                    ]
                    src = ps.rearrange("p (r c) -> p r c", c=W)
                if (m + t) % 2 == 0:
                    nc.scalar.activation(
                        out=dest, in_=src,
                        func=mybir.ActivationFunctionType.Identity,
                        bias=bq_sb[:, m:m + 1], scale=1.0,
                    )
                else:
                    nc.vector.tensor_scalar_add(dest, src, bq_sb[:, m:m + 1])

        # ---- phase 2: scores + softmax ----
        for br in range(3):
            d = DILS[br]
            Kg = K_sb[:, br, :].rearrange("p (r c) -> p r c", c=PW)
            for t in range(NBLK):
                sl = slice(t * BLK, (t + 1) * BLK)
                sp = ppb.tile([18, BLK], F32, tag="s18")
                q_in = Q_sb[:, br, sl].rearrange("p (r c) -> p r c", c=W)
                for j, (a, b) in enumerate(SHIFTS):
                    prod = prodp.tile([96, 8, W], BF16, tag="prod")
                    k_in = Kg[:, 8 * t + PADR + a * d: 8 * t + 8 + PADR + a * d,
                              PADC + b * d: PADC + b * d + W]
                    nc.vector.tensor_tensor(prod, q_in, k_in, op=AluOpType.mult)
                    nc.tensor.matmul(
                        sp,
                        lhsT=ones18_sb[:, j, :],
                        rhs=prod.rearrange("p r c -> p (r c)"),
                        start=(j == 0), stop=(j == 8),
                    )
                # exp (scale folded in)
                nc.scalar.activation(
                    out=Pc_sb[:, br, sl], in_=sp,
                    func=mybir.ActivationFunctionType.Exp, scale=SCALE,
                )
                # denominator, born replicated over the 18 rows
                dp = ppb.tile([18, BLK], F32, tag="s18")
                nc.tensor.matmul(dp, lhsT=selden_sb, rhs=Pc_sb[:, br, sl],
                                 start=True, stop=True)
                rec = smallp.tile([18, BLK], F32, tag="rec")
                nc.vector.reciprocal(rec, dp)
                nc.vector.tensor_tensor(Pn_sb[:, br, sl], Pc_sb[:, br, sl], rec,
                                        op=AluOpType.mult)

        # ---- phase 3: attention-weighted V ----
        for br in range(3):
            d = DILS[br]
            Vg = V_sb[:, br, :].rearrange("p (r c) -> p r c", c=PW)
            for t in range(NBLK):
                sl = slice(t * BLK, (t + 1) * BLK)
                prj = prjp.tile([96, BLK, 9], BF16, tag="prj")
                prjg = prj.rearrange("p (r c) j -> p r c j", r=8)
                for j, (a, b) in enumerate(SHIFTS):
                    pb = ppa.tile([96, BLK], F32, tag="mm96")
                    nc.tensor.matmul(pb, lhsT=selj_sb[:, j, :],
                                     rhs=Pn_sb[:, br, sl], start=True, stop=True)
                    v_in = Vg[:, 8 * t + PADR + a * d: 8 * t + 8 + PADR + a * d,
                              PADC + b * d: PADC + b * d + W]
                    nc.vector.scalar_tensor_tensor(
                        out=prjg[:, :, :, j],
                        in0=pb.rearrange("p (r c) -> p r c", c=W),
                        scalar=1.0,
                        in1=v_in,
                        op0=AluOpType.bypass,
                        op1=AluOpType.mult,
                    )
                with nc.allow_low_precision(reason="9-tap sum; fp32 ALU, one bf16 round"):
                    nc.vector.tensor_reduce(
                        out=yb_sb[:, br, sl], in_=prj,
                        axis=mybir.AxisListType.X, op=AluOpType.add,
                    )

        # ---- phase 4: output projection + write out ----
        for t in range(25):
            size = min(128, NPIX - t * 128)
            py = ppa.tile([128, C], F32, tag="mmproj")
            for br in range(3):
                nc.tensor.matmul(
                    py[:size, :],
                    lhsT=yb_sb[:, br, t * 128:t * 128 + size],
                    rhs=wp_sb[:, br, :],
                    start=(br == 0), stop=(br == 2),
                )
            yo_t = smallp.tile([128, C], F32, tag="yo")
            nc.vector.tensor_tensor(yo_t[:size, :], py[:size, :],
                                    bp_sb[:size, :], op=AluOpType.add)
            nc.sync.dma_start(out=y[t * 128:t * 128 + size, :],
                              in_=yo_t[:size, :])

    nc.compile()
    return nc


def host_inputs(x, w_qkv, b_qkv, w_proj, b_proj):
    """Numpy prep of per-core + shared input arrays (keys match dram names)."""
    x = np.asarray(x, np.float32)
    w_qkv = np.asarray(w_qkv, np.float32)
    b_qkv = np.asarray(b_qkv, np.float32)
    w_proj = np.asarray(w_proj, np.float32)
    b_proj = np.asarray(b_proj, np.float32)

    # xT per core: [96, 3, NPIX]
    xT_all = x.reshape(B, NPIX, C).transpose(0, 2, 1)          # [B, C, NPIX]
    xT_all = xT_all.reshape(B, 3, 96, NPIX).transpose(0, 2, 1, 3)  # [B,96,3,NPIX]
    xT_all = np.ascontiguousarray(xT_all).astype(NPBF16)

    # wq: [cin96, m, k, cout96] = w_qkv[m*96+cout, k*96+cin]
    w3 = w_qkv.reshape(9, 96, 3, 96)                            # [m,cout,k,cin]
    wq_h = np.ascontiguousarray(w3.transpose(3, 0, 2, 1)).astype(NPBF16)
    bq_h = np.ascontiguousarray(b_qkv.reshape(9, 96).T).astype(np.float32)

    # wp: [cin96, branch, cout] = w_proj[cout, branch*96+cin]
    wp_h = np.ascontiguousarray(
        w_proj.reshape(C, 3, 96).transpose(2, 1, 0)).astype(NPBF16)
    bp_h = np.ascontiguousarray(
        np.broadcast_to(b_proj[None, :], (128, C))).astype(np.float32)

    ones18_h = np.zeros((96, 9, 18), NPBF16)
    for j in range(9):
        ones18_h[0:48, j, 2 * j] = 1
        ones18_h[48:96, j, 2 * j + 1] = 1

    selden_h = np.zeros((18, 18), NPBF16)
    for j in range(9):
        for h in range(2):
            for j2 in range(9):
                selden_h[2 * j + h, 2 * j2 + h] = 1

    selj_h = np.zeros((18, 9, 96), NPBF16)
    for j in range(9):
        for h in range(2):
            selj_h[2 * j + h, j, h * 48:(h + 1) * 48] = 1


    shared = dict(wq=wq_h, bq=bq_h, wp=wp_h, bp=bp_h, ones18=ones18_h,
                  selden=selden_h, selj=selj_h)
    in_maps = [dict(shared, xT=np.ascontiguousarray(xT_all[i]))
               for i in range(N_CORES)]
    return in_maps


_NC_CACHE = {}


def kernel(x, w_qkv, b_qkv, w_proj, b_proj):
    if "nc" not in _NC_CACHE:
        _NC_CACHE["nc"] = build_nc()
    nc = _NC_CACHE["nc"]
    in_maps = host_inputs(x, w_qkv, b_qkv, w_proj, b_proj)
    res = run_bass_kernel_spmd(nc, in_maps, list(range(N_CORES)))
    out = np.stack([res.results[i]["y"] for i in range(N_CORES)], axis=0)
    return out.reshape(B, H, W, C).astype(np.float32)


if __name__ == "__main__":
    rng = np.random.default_rng(0)
    xs = rng.standard_normal((B, H, W, C), dtype=np.float32)
    print("built nc ok" if build_nc() else "")

